# revision 36
# baseline (speedup 1.0000x reference)
"""ChannelMHSA on Trainium2 (Bass/Tile), data-parallel over batch on 8 cores.

Reference computation (per batch b of x [N, C]):
    qkv  = x @ w_qkv                      # [N, 3C]
    q, k, v per head h: [N, D]
    z_h  = k_h^T @ v_h / sqrt(D)          # [D, D]
    A_h  = softmax(z_h, axis=-1)
    out[n, (h,d)] = sum_e A_h[d, e] q[n, (h,e)]
    y    = out @ w_out                    # [N, C]

Restructured algebra (k, v only ever appear inside z; the output path is
linear in A), which cuts PE work ~25% vs the direct formulation and needs
no on-device transposes:
    G  = x^T @ x                          # [C, C]
    ST = G @ w_k                          # [C, C]   (uses G^T = G)
    z_h  = ST[:, h]^T @ w_v[:, h]         # [D, D] per head, packed in pairs
    A_h  = softmax(z_h / 8)
    R[(h,e), :] = sum_d A_h[d, e] * w_out[(h,d), :]   # block-diag lhsT trick
    P  = w_q @ R                          # [C, C]
    y  = x @ P                            # [N, C]

All matmul operands are bf16 (1 PE cycle/row at any free size, FWL weight
loads, half DMA); PSUM accumulation is fp32; measured rel err ~1e-2 vs the
2e-2 gate.  x^T, w_q^T and all casts are prepared host-side in make_in_maps.

Scheduling: the per-pair softmax (exp on ACT -> reciprocal/block-diag fill
on DVE) has ~1.5-2us of serial latency that would bubble the PE between the
tiny z and R matmuls.  The pair loop therefore takes a "filler" work list:
batch 0 weaves the next batch's G chains in; later batches weave 12
held-back y chains of the previous batch.  A ~3us dummy-matmul prewarm
lifts the PE HAM clock gate to 8/8 before real work arrives.  Inputs and
outputs stream on the Sync hardware-DGE queue (GpSimd's software DGE is
~5x slower and only carries half the batch-0 x load plus part of the
final-tile drain, where queue parallelism still helps).
"""

import sys
from contextlib import ExitStack

import numpy as np

for _p in ("/opt/trn_rl_repo", "/opt/pypackages"):
    if _p not in sys.path:
        sys.path.append(_p)

import concourse.bacc as bacc
import concourse.mybir as mybir
import concourse.tile as tile
from concourse import bass_utils, masks

B, N, C = 32, 1024, 768
H, D = 12, 64
P = 128
NCORES = 8
BS = B // NCORES          # batches per core
KC = C // P               # 6 chunks over C
NM = N // P               # 8 chunks over N
NPAIR = H // 2            # 6 head pairs
F32 = mybir.dt.float32
BF16 = mybir.dt.bfloat16
FH = 384                  # free-dim half of C for matmul tiling
NHELD = 12                # y chains of batch b woven into batch b+1's pairs


def _emit(ctx, tc, x_d, xt_d, wk_d, wv_d, wo_d, wqt_d, y_d):
    nc = tc.nc

    const = ctx.enter_context(tc.tile_pool(name="const", bufs=1))
    xin_pool = ctx.enter_context(tc.tile_pool(name="xin", bufs=2 * NM))
    xt_pool = ctx.enter_context(tc.tile_pool(name="xtp", bufs=3))
    g_pool = ctx.enter_context(tc.tile_pool(name="gp", bufs=KC + 2))
    st_pool = ctx.enter_context(tc.tile_pool(name="stp", bufs=KC + 2))
    r_pool = ctx.enter_context(tc.tile_pool(name="rp", bufs=2))
    p_pool = ctx.enter_context(tc.tile_pool(name="pp", bufs=2))
    y_pool = ctx.enter_context(tc.tile_pool(name="yp", bufs=8))
    sm_pool = ctx.enter_context(tc.tile_pool(name="smp", bufs=8))
    psG = ctx.enter_context(tc.tile_pool(name="psG", bufs=6, space="PSUM"))
    psZ = ctx.enter_context(tc.tile_pool(name="psZ", bufs=2, space="PSUM"))

    ci = [0]

    def copy_out(dst, src):
        # alternate PSUM->SBUF copies between DVE and ACT to balance load
        if ci[0] % 2 == 0:
            nc.vector.tensor_copy(dst, src)
        else:
            nc.scalar.copy(dst, src)
        ci[0] += 1

    # ---- PE pre-warm: ~3us of dummy matmuls on a memset tile so the HAM
    # clock gate reaches 8/8 and the array is hot right as batch-0 x lands;
    # they depend on nothing but the memset so they start with the kernel ----
    warmz = const.tile([P, P], BF16, tag="warmz", name="warmz")
    nc.vector.memset(warmz[:], 0.0)
    wps = psZ.tile([P, P], F32, tag="z", name="warmps", space="PSUM")
    for i in range(30):
        nc.tensor.matmul(wps[:], warmz[:], warmz[:], start=(i == 0),
                         stop=(i == 29))

    # ---- input DMAs; batch 0's x is split finely across both queues to
    # minimize the PE's cold-start wait ----
    xin_b, xt_b = {}, {}

    def emit_x_dmas(b, split=False):
        xin = [xin_pool.tile([P, C], BF16, tag="xin", name=f"xin{b}_{m}")
               for m in range(NM)]
        for m in range(NM):
            if split:
                nc.sync.dma_start(xin[m][:, :FH],
                                  x_d[b, m * P:(m + 1) * P, :FH])
                nc.gpsimd.dma_start(xin[m][:, FH:],
                                    x_d[b, m * P:(m + 1) * P, FH:])
            else:
                nc.sync.dma_start(xin[m][:], x_d[b, m * P:(m + 1) * P, :])
        xt = xt_pool.tile([P, KC * N], BF16, tag="xt", name=f"xt{b}")
        nc.sync.dma_start(xt[:], xt_d[b])
        xin_b[b], xt_b[b] = xin, xt

    emit_x_dmas(0, split=True)

    # weights ordered by first use: wk (ST), wv (z), wo (R), wqt (P)
    wk, wv, wo = [], [], []
    for k in range(KC):
        t = const.tile([P, C], BF16, tag=f"wk{k}", name=f"wk{k}")
        nc.sync.dma_start(t[:], wk_d[k * P:(k + 1) * P, :])
        wk.append(t)
    for k in range(KC):
        t = const.tile([P, C], BF16, tag=f"wv{k}", name=f"wv{k}")
        nc.sync.dma_start(t[:], wv_d[k * P:(k + 1) * P, :])
        wv.append(t)
    for k in range(KC):
        t = const.tile([P, C], BF16, tag=f"wo{k}", name=f"wo{k}")
        nc.sync.dma_start(t[:], wo_d[k * P:(k + 1) * P, :])
        wo.append(t)
    wqt = const.tile([P, KC * C], BF16, tag="wqt", name="wqt")
    nc.sync.dma_start(wqt[:], wqt_d[:])

    emit_x_dmas(1)

    # Two persistent block-diag lhsT tiles for the R matmul; only the two
    # diagonal [64,64] blocks are rewritten per pair, off-diag zeros persist.
    zeros = const.tile([P, P], F32, tag="zeros", name="zeros")
    nc.vector.memset(zeros[:], 0.0)
    bd_tiles = []
    for i in range(3):
        t = const.tile([P, P], BF16, tag=f"bd{i}", name=f"bd{i}")
        nc.vector.tensor_copy(t[:], zeros[:])
        bd_tiles.append(t)
    ident = const.tile([P, P], BF16, tag="ident", name="ident")
    masks.make_identity(nc, ident[:])

    G_b, ST_b, r_b, p_b = {}, {}, {}, {}

    # ---- phase emitters, shaped as thunk lists so they can be woven ----
    def g_thunks(b):
        """G = x^T x.  Only the upper-triangle blocks (col chunk >= row
        chunk) are computed by matmul; the 15 lower blocks are PE-transposed
        copies of their mirror.  ~40% fewer G cycles."""
        xin = xin_b[b]
        G = [g_pool.tile([P, C], BF16, tag="G", name=f"G{b}_{k}")
             for k in range(KC)]
        G_b[b] = G

        def mk_chain(po, cs, w):
            def emit():
                ps = psG.tile([P, w], F32, tag="ps",
                              name=f"psg{b}_{po}_{cs}", space="PSUM")
                for m in range(NM):
                    nc.tensor.matmul(
                        ps[:],
                        xin[m][:, po * P:(po + 1) * P],
                        xin[m][:, cs:cs + w],
                        start=(m == 0), stop=(m == NM - 1))
                copy_out(G[po][:, cs:cs + w], ps[:])
            return emit

        def mk_mirror(blocks):
            def emit():
                for po, kk in blocks:
                    tp = psG.tile([P, P], BF16, tag="ps",
                                  name=f"tpg{b}_{po}_{kk}", space="PSUM")
                    nc.tensor.transpose(tp[:], G[kk][:, po * P:(po + 1) * P],
                                        ident[:])
                    copy_out(G[po][:, kk * P:(kk + 1) * P], tp[:])
            return emit

        thunks = []
        for po in range(KC):
            cs = po * P
            while cs < C:
                rem = C - cs
                # a 512-wide chunk fills one PSUM bank exactly and beats
                # a 384+128 split on per-matmul issue overhead
                w = 512 if rem == 512 else min(FH, rem)
                thunks.append(mk_chain(po, cs, w))
                cs += w
        lower = [(po, kk) for po in range(KC) for kk in range(po)]
        for i in range(0, len(lower), 5):
            thunks.append(mk_mirror(lower[i:i + 5]))
        return thunks

    def emit_st(b):
        G = G_b.pop(b)
        ST = [st_pool.tile([P, C], BF16, tag="ST", name=f"ST{b}_{k}")
              for k in range(KC)]
        ST_b[b] = ST
        for po in range(KC):
            for f in range(2):
                ps = psG.tile([P, FH], F32, tag="ps", name=f"pss{b}_{po}_{f}",
                              space="PSUM")
                for k in range(KC):
                    nc.tensor.matmul(
                        ps[:],
                        G[k][:, po * P:(po + 1) * P],
                        wk[k][:, f * FH:(f + 1) * FH],
                        start=(k == 0), stop=(k == KC - 1))
                copy_out(ST[po][:, f * FH:(f + 1) * FH], ps[:])

    def emit_pairs(b, filler):
        """z -> softmax -> R per head pair, pulling filler thunks in to keep
        the PE fed while exp/reciprocal run on ACT/DVE."""
        ST = ST_b.pop(b)
        r16 = r_pool.tile([P, KC * C], BF16, tag="r16", name=f"r16{b}")
        r_b[b] = r16
        zps_pair = {}
        for step in range(NPAIR + 2):
            if step == 0 and filler:
                # cover the ST-copy latency at the phase boundary
                filler.pop(0)()
            if step < NPAIR:
                pr = step
                zps = psZ.tile([P, P], F32, tag="z", name=f"z{b}_{pr}",
                               space="PSUM")
                zps_pair[pr] = zps
                for k in range(KC):
                    nc.tensor.matmul(
                        zps[:],
                        ST[k][:, pr * P:(pr + 1) * P],
                        wv[k][:, pr * P:(pr + 1) * P],
                        start=(k == 0), stop=(k == KC - 1))
            # the late steps have little-to-no z work left to hide the
            # softmax serial latency, so give them double filler
            npop = 2 if (step == 0 or step >= NPAIR - 1) else 1
            for _ in range(npop):
                if filler:
                    filler.pop(0)()
            if 1 <= step <= NPAIR:
                # softmax for pair (step-1); its R matmul is deferred one
                # more step so the block-diag LDWEIGHTS never waits on DVE
                pr = step - 1
                zps = zps_pair.pop(pr)
                bdt = bd_tiles[pr % 3]
                ssum = sm_pool.tile([P, 1], F32, tag="ssum", name=f"ss{b}_{pr}")
                aexs = []
                for j in range(2):
                    rb = j * D
                    aex = sm_pool.tile([P, D], F32, tag="aex",
                                       name=f"ae{b}_{pr}_{j}")
                    aexs.append(aex)
                    # softmax needs no max-shift: |z/8| <= ~25, fp32-exp safe
                    nc.scalar.activation(aex[rb:rb + D, :],
                                         zps[rb:rb + D, rb:rb + D],
                                         mybir.ActivationFunctionType.Exp,
                                         bias=0.0, scale=0.125,
                                         accum_out=ssum[rb:rb + D, :])
                rinv = sm_pool.tile([P, 1], F32, tag="rinv",
                                    name=f"ri{b}_{pr}")
                nc.vector.reciprocal(rinv[:], ssum[:])
                for j in range(2):
                    rb = j * D
                    nc.vector.tensor_scalar_mul(bdt[rb:rb + D, rb:rb + D],
                                                aexs[j][rb:rb + D, :],
                                                rinv[rb:rb + D, :])
            if step >= 2:
                pr2 = step - 2
                bdt2 = bd_tiles[pr2 % 3]
                for f in range(2):
                    ps = psG.tile([P, FH], F32, tag="ps",
                                  name=f"psr{b}_{pr2}_{f}", space="PSUM")
                    nc.tensor.matmul(ps[:], bdt2[:],
                                     wo[pr2][:, f * FH:(f + 1) * FH],
                                     start=True, stop=True)
                    # keep the pair phase's copies off ACT (busy with exp)
                    nc.vector.tensor_copy(
                        r16[:, pr2 * C + f * FH:pr2 * C + (f + 1) * FH],
                        ps[:])
        while filler:
            filler.pop(0)()

    def emit_p(b):
        r16 = r_b.pop(b)
        p16 = p_pool.tile([P, KC * C], BF16, tag="p16", name=f"p16{b}")
        p_b[b] = p16
        for po in range(KC):
            for f in range(2):
                ps = psG.tile([P, FH], F32, tag="ps", name=f"psp{b}_{po}_{f}",
                              space="PSUM")
                for k in range(KC):
                    nc.tensor.matmul(
                        ps[:],
                        wqt[:, k * C + po * P:k * C + (po + 1) * P],
                        r16[:, k * C + f * FH:k * C + (f + 1) * FH],
                        start=(k == 0), stop=(k == KC - 1))
                copy_out(p16[:, po * C + f * FH:po * C + (f + 1) * FH], ps[:])

    def y_thunks(b):
        xt, p16 = xt_b[b], p_b[b]
        yts = {}

        def mk(m, f):
            def emit():
                if m not in yts:
                    yts[m] = y_pool.tile([P, C], F32, tag="y", name=f"y{b}_{m}")
                yt = yts[m]
                ps = psG.tile([P, FH], F32, tag="ps", name=f"psy{b}_{m}_{f}",
                              space="PSUM")
                for k in range(KC):
                    nc.tensor.matmul(
                        ps[:],
                        xt[:, k * N + m * P:k * N + (m + 1) * P],
                        p16[:, k * C + f * FH:k * C + (f + 1) * FH],
                        start=(k == 0), stop=(k == KC - 1))
                copy_out(yt[:, f * FH:(f + 1) * FH], ps[:])
                if b == BS - 1 and m >= NM - 3:
                    # very end of the kernel: quarter the output across four
                    # DMA queues so the drain doesn't trail the compute
                    h = FH // 2
                    for q, eng in enumerate((nc.sync, nc.gpsimd)):
                        cs = f * FH + q * h
                        eng.dma_start(
                            y_d[b, m * P:(m + 1) * P, cs:cs + h],
                            yt[:, cs:cs + h])
                elif b == BS - 1:
                    eng = nc.sync if (2 * m + f) % 2 == 0 else nc.gpsimd
                    eng.dma_start(
                        y_d[b, m * P:(m + 1) * P, f * FH:(f + 1) * FH],
                        yt[:, f * FH:(f + 1) * FH])
                elif f == 1:
                    nc.sync.dma_start(y_d[b, m * P:(m + 1) * P, :], yt[:])
            return emit
        return [mk(m, f) for m in range(NM) for f in range(2)]

    # ---- driver ----
    for t in g_thunks(0):
        t()
    emit_st(0)
    held = []
    for b in range(BS):
        filler = g_thunks(1) if b == 0 else held
        emit_pairs(b, filler)
        emit_p(b)
        if b + 2 < BS:
            emit_x_dmas(b + 2)
        if 0 < b < BS - 1:
            for t in g_thunks(b + 1):
                t()
        yth = y_thunks(b)
        if b < BS - 1:
            for t in yth[:-NHELD]:
                t()
            held = yth[-NHELD:]
            emit_st(b + 1)
        else:
            for t in yth:
                t()


_BUILD_CACHE = {}


def build_program():
    if "nc" in _BUILD_CACHE:
        return _BUILD_CACHE["nc"]
    nc = bacc.Bacc("TRN2", target_bir_lowering=False, debug=False,
                   num_devices=NCORES)
    x_d = nc.dram_tensor("x16", [BS, N, C], BF16, kind="ExternalInput").ap()
    xt_d = nc.dram_tensor("xt16", [BS, P, KC * N], BF16,
                          kind="ExternalInput").ap()
    wk_d = nc.dram_tensor("wk", [C, C], BF16, kind="ExternalInput").ap()
    wv_d = nc.dram_tensor("wv", [C, C], BF16, kind="ExternalInput").ap()
    wo_d = nc.dram_tensor("wo", [C, C], BF16, kind="ExternalInput").ap()
    wqt_d = nc.dram_tensor("wqt", [P, KC * C], BF16, kind="ExternalInput").ap()
    y_d = nc.dram_tensor("y", [BS, N, C], F32, kind="ExternalOutput").ap()
    with tile.TileContext(nc) as tc:
        with ExitStack() as ctx:
            _emit(ctx, tc, x_d, xt_d, wk_d, wv_d, wo_d, wqt_d, y_d)
    nc.compile()
    _BUILD_CACHE["nc"] = nc
    return nc


def make_in_maps(x, w_qkv, w_out):
    import ml_dtypes
    bf16 = ml_dtypes.bfloat16
    x = np.asarray(x, dtype=np.float32)
    w_qkv = np.asarray(w_qkv, dtype=np.float32)
    w_out = np.asarray(w_out, dtype=np.float32)

    x16 = np.ascontiguousarray(x.astype(bf16))                    # [B, N, C]
    # xt[b, p, k*N + n] = x[b, n, k*128 + p]
    xt = np.ascontiguousarray(
        x16.transpose(0, 2, 1).reshape(B, KC, P, N)
           .transpose(0, 2, 1, 3).reshape(B, P, KC * N))
    # wqt[p, k*C + c] = w_q[c, k*128 + p]
    wqt = np.ascontiguousarray(
        w_qkv[:, :C].T.reshape(KC, P, C).transpose(1, 0, 2)
                     .reshape(P, KC * C).astype(bf16))
    wk = np.ascontiguousarray(w_qkv[:, C:2 * C].astype(bf16))
    wv = np.ascontiguousarray(w_qkv[:, 2 * C:].astype(bf16))
    wo16 = np.ascontiguousarray(w_out.astype(bf16))
    return [
        {"x16": x16[i * BS:(i + 1) * BS], "xt16": xt[i * BS:(i + 1) * BS],
         "wk": wk, "wv": wv, "wo": wo16, "wqt": wqt}
        for i in range(NCORES)
    ]


def kernel(x, w_qkv, b_qkv=None, w_out=None, b_out=None, **_unused):
    nc = build_program()
    in_maps = make_in_maps(x, w_qkv, w_out)
    res = bass_utils.run_bass_kernel_spmd(nc, in_maps,
                                          core_ids=list(range(NCORES)))
    y = np.concatenate([res.results[i]["y"] for i in range(NCORES)], axis=0)
    return np.asarray(y, dtype=np.float32)


# revision 38
# speedup vs baseline: 1.0224x; 1.0224x over previous
"""ChannelMHSA on Trainium2 (Bass/Tile), data-parallel over batch on 8 cores.

Reference computation (per batch b of x [N, C]):
    qkv  = x @ w_qkv                      # [N, 3C]
    q, k, v per head h: [N, D]
    z_h  = k_h^T @ v_h / sqrt(D)          # [D, D]
    A_h  = softmax(z_h, axis=-1)
    out[n, (h,d)] = sum_e A_h[d, e] q[n, (h,e)]
    y    = out @ w_out                    # [N, C]

Restructured algebra (k, v only ever appear inside z; the output path is
linear in A), which cuts PE work ~25% vs the direct formulation and needs
no on-device transposes:
    G  = x^T @ x                          # [C, C]
    ST = G @ w_k                          # [C, C]   (uses G^T = G)
    z_h  = ST[:, h]^T @ w_v[:, h]         # [D, D] per head, packed in pairs
    A_h  = softmax(z_h / 8)
    R[(h,e), :] = sum_d A_h[d, e] * w_out[(h,d), :]   # block-diag lhsT trick
    P  = w_q @ R                          # [C, C]
    y  = x @ P                            # [N, C]

All matmul operands are bf16 (1 PE cycle/row at any free size, FWL weight
loads, half DMA); PSUM accumulation is fp32; measured rel err ~1e-2 vs the
2e-2 gate.  x^T, w_q^T and all casts are prepared host-side in make_in_maps.

Scheduling: the per-pair softmax (exp on ACT -> reciprocal/block-diag fill
on DVE) has ~1.5-2us of serial latency that would bubble the PE between the
tiny z and R matmuls.  The pair loop therefore takes a "filler" work list:
batch 0 weaves the next batch's G chains in; later batches weave 12
held-back y chains of the previous batch.  A ~3us dummy-matmul prewarm
lifts the PE HAM clock gate to 8/8 before real work arrives.  Inputs and
outputs stream on the Sync hardware-DGE queue (GpSimd's software DGE is
~5x slower and only carries half the batch-0 x load plus part of the
final-tile drain, where queue parallelism still helps).
"""

import sys
from contextlib import ExitStack

import numpy as np

for _p in ("/opt/trn_rl_repo", "/opt/pypackages"):
    if _p not in sys.path:
        sys.path.append(_p)

import concourse.bacc as bacc
import concourse.mybir as mybir
import concourse.tile as tile
from concourse import bass_utils, masks

B, N, C = 32, 1024, 768
H, D = 12, 64
P = 128
NCORES = 8
BS = B // NCORES          # batches per core
KC = C // P               # 6 chunks over C
NM = N // P               # 8 chunks over N
NPAIR = H // 2            # 6 head pairs
F32 = mybir.dt.float32
BF16 = mybir.dt.bfloat16
FH = 384                  # free-dim half of C for matmul tiling
NHELD = 12                # y chains of batch b woven into batch b+1's pairs


def _emit(ctx, tc, x_d, xt_d, wk_d, wv_d, wo_d, wqt_d, y_d):
    nc = tc.nc

    const = ctx.enter_context(tc.tile_pool(name="const", bufs=1))
    xin_pool = ctx.enter_context(tc.tile_pool(name="xin", bufs=2 * NM))
    xt_pool = ctx.enter_context(tc.tile_pool(name="xtp", bufs=3))
    g_pool = ctx.enter_context(tc.tile_pool(name="gp", bufs=KC + 2))
    st_pool = ctx.enter_context(tc.tile_pool(name="stp", bufs=KC + 2))
    r_pool = ctx.enter_context(tc.tile_pool(name="rp", bufs=2))
    p_pool = ctx.enter_context(tc.tile_pool(name="pp", bufs=2))
    y_pool = ctx.enter_context(tc.tile_pool(name="yp", bufs=8))
    sm_pool = ctx.enter_context(tc.tile_pool(name="smp", bufs=8))
    psG = ctx.enter_context(tc.tile_pool(name="psG", bufs=6, space="PSUM"))
    psZ = ctx.enter_context(tc.tile_pool(name="psZ", bufs=2, space="PSUM"))

    ci = [0]

    def copy_out(dst, src):
        # alternate PSUM->SBUF copies between DVE and ACT to balance load
        if ci[0] % 2 == 0:
            nc.vector.tensor_copy(dst, src)
        else:
            nc.scalar.copy(dst, src)
        ci[0] += 1

    # ---- PE pre-warm: ~3us of dummy matmuls on a memset tile so the HAM
    # clock gate reaches 8/8 and the array is hot right as batch-0 x lands;
    # they depend on nothing but the memset so they start with the kernel ----
    warmz = const.tile([P, P], BF16, tag="warmz", name="warmz")
    nc.vector.memset(warmz[:], 0.0)
    wps = psZ.tile([P, P], F32, tag="z", name="warmps", space="PSUM")
    for i in range(30):
        nc.tensor.matmul(wps[:], warmz[:], warmz[:], start=(i == 0),
                         stop=(i == 29))

    # ---- input DMAs; batch 0's x is split finely across both queues to
    # minimize the PE's cold-start wait ----
    xin_b, xt_b = {}, {}

    def emit_x_dmas(b, split=False):
        xin = [xin_pool.tile([P, C], BF16, tag="xin", name=f"xin{b}_{m}")
               for m in range(NM)]
        for m in range(NM):
            if split:
                nc.sync.dma_start(xin[m][:, :FH],
                                  x_d[b, m * P:(m + 1) * P, :FH])
                nc.gpsimd.dma_start(xin[m][:, FH:],
                                    x_d[b, m * P:(m + 1) * P, FH:])
            else:
                nc.sync.dma_start(xin[m][:], x_d[b, m * P:(m + 1) * P, :])
        xt = xt_pool.tile([P, KC * N], BF16, tag="xt", name=f"xt{b}")
        nc.sync.dma_start(xt[:], xt_d[b])
        xin_b[b], xt_b[b] = xin, xt

    emit_x_dmas(0, split=True)

    # weights ordered by first use: wk (ST), wv (z), wo (R), wqt (P)
    wk, wv, wo = [], [], []
    for k in range(KC):
        t = const.tile([P, C], BF16, tag=f"wk{k}", name=f"wk{k}")
        nc.sync.dma_start(t[:], wk_d[k * P:(k + 1) * P, :])
        wk.append(t)
    for k in range(KC):
        t = const.tile([P, C], BF16, tag=f"wv{k}", name=f"wv{k}")
        nc.sync.dma_start(t[:], wv_d[k * P:(k + 1) * P, :])
        wv.append(t)
    for k in range(KC):
        t = const.tile([P, C], BF16, tag=f"wo{k}", name=f"wo{k}")
        nc.sync.dma_start(t[:], wo_d[k * P:(k + 1) * P, :])
        wo.append(t)
    wqt = const.tile([P, KC * C], BF16, tag="wqt", name="wqt")
    nc.sync.dma_start(wqt[:], wqt_d[:])

    emit_x_dmas(1)

    # Two persistent block-diag lhsT tiles for the R matmul; only the two
    # diagonal [64,64] blocks are rewritten per pair, off-diag zeros persist.
    zeros = const.tile([P, P], F32, tag="zeros", name="zeros")
    nc.vector.memset(zeros[:], 0.0)
    bd_tiles = []
    for i in range(3):
        t = const.tile([P, P], BF16, tag=f"bd{i}", name=f"bd{i}")
        nc.vector.tensor_copy(t[:], zeros[:])
        bd_tiles.append(t)
    ident = const.tile([P, P], BF16, tag="ident", name="ident")
    masks.make_identity(nc, ident[:])

    G_b, ST_b, r_b, p_b = {}, {}, {}, {}

    # ---- phase emitters, shaped as thunk lists so they can be woven ----
    def g_thunks(b):
        """G = x^T x.  Only the upper-triangle blocks (col chunk >= row
        chunk) are computed by matmul; the 15 lower blocks are PE-transposed
        copies of their mirror.  ~40% fewer G cycles."""
        xin = xin_b[b]
        G = [g_pool.tile([P, C], BF16, tag="G", name=f"G{b}_{k}")
             for k in range(KC)]
        G_b[b] = G

        def mk_chain(po, cs, w):
            def emit():
                ps = psG.tile([P, FH], F32, tag="ps",
                              name=f"psg{b}_{po}_{cs}", space="PSUM")
                for m in range(NM):
                    nc.tensor.matmul(
                        ps[:, :w],
                        xin[m][:, po * P:(po + 1) * P],
                        xin[m][:, cs:cs + w],
                        start=(m == 0), stop=(m == NM - 1))
                copy_out(G[po][:, cs:cs + w], ps[:, :w])
            return emit

        def mk_mirror(blocks):
            def emit():
                for po, kk in blocks:
                    tp = psG.tile([P, P], BF16, tag="ps",
                                  name=f"tpg{b}_{po}_{kk}", space="PSUM")
                    nc.tensor.transpose(tp[:], G[kk][:, po * P:(po + 1) * P],
                                        ident[:])
                    copy_out(G[po][:, kk * P:(kk + 1) * P], tp[:])
            return emit

        thunks = []
        for po in range(KC):
            cs = po * P
            while cs < C:
                w = min(FH, C - cs)
                thunks.append(mk_chain(po, cs, w))
                cs += w
        lower = [(po, kk) for po in range(KC) for kk in range(po)]
        for i in range(0, len(lower), 5):
            thunks.append(mk_mirror(lower[i:i + 5]))
        return thunks

    def emit_st(b, hold=0):
        G = G_b.pop(b)
        ST = [st_pool.tile([P, C], BF16, tag="ST", name=f"ST{b}_{k}")
              for k in range(KC)]
        ST_b[b] = ST

        def mk(po, f):
            def emit():
                ps = psG.tile([P, FH], F32, tag="ps", name=f"pss{b}_{po}_{f}",
                              space="PSUM")
                for k in range(KC):
                    nc.tensor.matmul(
                        ps[:],
                        G[k][:, po * P:(po + 1) * P],
                        wk[k][:, f * FH:(f + 1) * FH],
                        start=(k == 0), stop=(k == KC - 1))
                copy_out(ST[po][:, f * FH:(f + 1) * FH], ps[:])
            return emit

        thunks = [mk(po, f) for po in range(KC) for f in range(2)]
        for t in thunks[:len(thunks) - hold]:
            t()
        return thunks[len(thunks) - hold:]

    def emit_pairs(b, filler):
        """z -> softmax -> R per head pair, pulling filler thunks in to keep
        the PE fed while exp/reciprocal run on ACT/DVE."""
        ST = ST_b.pop(b)
        r16 = r_pool.tile([P, KC * C], BF16, tag="r16", name=f"r16{b}")
        r_b[b] = r16
        zps_pair = {}
        for step in range(NPAIR + 2):
            if step == 0 and filler:
                # cover the ST-copy latency at the phase boundary
                filler.pop(0)()
            if step < NPAIR:
                pr = step
                zps = psZ.tile([P, P], F32, tag="z", name=f"z{b}_{pr}",
                               space="PSUM")
                zps_pair[pr] = zps
                for k in range(KC):
                    nc.tensor.matmul(
                        zps[:],
                        ST[k][:, pr * P:(pr + 1) * P],
                        wv[k][:, pr * P:(pr + 1) * P],
                        start=(k == 0), stop=(k == KC - 1))
            # the late steps have little-to-no z work left to hide the
            # softmax serial latency, so give them double filler
            npop = 2 if (step == 0 or step >= NPAIR - 1) else 1
            for _ in range(npop):
                if filler:
                    filler.pop(0)()
            if 1 <= step <= NPAIR:
                # softmax for pair (step-1); its R matmul is deferred one
                # more step so the block-diag LDWEIGHTS never waits on DVE
                pr = step - 1
                zps = zps_pair.pop(pr)
                bdt = bd_tiles[pr % 3]
                ssum = sm_pool.tile([P, 1], F32, tag="ssum", name=f"ss{b}_{pr}")
                aexs = []
                for j in range(2):
                    rb = j * D
                    aex = sm_pool.tile([P, D], F32, tag="aex",
                                       name=f"ae{b}_{pr}_{j}")
                    aexs.append(aex)
                    # softmax needs no max-shift: |z/8| <= ~25, fp32-exp safe
                    nc.scalar.activation(aex[rb:rb + D, :],
                                         zps[rb:rb + D, rb:rb + D],
                                         mybir.ActivationFunctionType.Exp,
                                         bias=0.0, scale=0.125,
                                         accum_out=ssum[rb:rb + D, :])
                rinv = sm_pool.tile([P, 1], F32, tag="rinv",
                                    name=f"ri{b}_{pr}")
                nc.vector.reciprocal(rinv[:], ssum[:])
                for j in range(2):
                    rb = j * D
                    nc.vector.tensor_scalar_mul(bdt[rb:rb + D, rb:rb + D],
                                                aexs[j][rb:rb + D, :],
                                                rinv[rb:rb + D, :])
            if step >= 2:
                pr2 = step - 2
                bdt2 = bd_tiles[pr2 % 3]
                for f in range(2):
                    ps = psG.tile([P, FH], F32, tag="ps",
                                  name=f"psr{b}_{pr2}_{f}", space="PSUM")
                    nc.tensor.matmul(ps[:], bdt2[:],
                                     wo[pr2][:, f * FH:(f + 1) * FH],
                                     start=True, stop=True)
                    # keep the pair phase's copies off ACT (busy with exp)
                    nc.vector.tensor_copy(
                        r16[:, pr2 * C + f * FH:pr2 * C + (f + 1) * FH],
                        ps[:])
        while filler:
            filler.pop(0)()

    def emit_p(b):
        r16 = r_b.pop(b)
        p16 = p_pool.tile([P, KC * C], BF16, tag="p16", name=f"p16{b}")
        p_b[b] = p16
        for po in range(KC):
            for f in range(2):
                ps = psG.tile([P, FH], F32, tag="ps", name=f"psp{b}_{po}_{f}",
                              space="PSUM")
                for k in range(KC):
                    nc.tensor.matmul(
                        ps[:],
                        wqt[:, k * C + po * P:k * C + (po + 1) * P],
                        r16[:, k * C + f * FH:k * C + (f + 1) * FH],
                        start=(k == 0), stop=(k == KC - 1))
                copy_out(p16[:, po * C + f * FH:po * C + (f + 1) * FH], ps[:])

    def y_thunks(b):
        xt, p16 = xt_b[b], p_b[b]
        yts = {}

        def mk(m, f):
            def emit():
                if m not in yts:
                    yts[m] = y_pool.tile([P, C], F32, tag="y", name=f"y{b}_{m}")
                yt = yts[m]
                ps = psG.tile([P, FH], F32, tag="ps", name=f"psy{b}_{m}_{f}",
                              space="PSUM")
                for k in range(KC):
                    nc.tensor.matmul(
                        ps[:],
                        xt[:, k * N + m * P:k * N + (m + 1) * P],
                        p16[:, k * C + f * FH:k * C + (f + 1) * FH],
                        start=(k == 0), stop=(k == KC - 1))
                copy_out(yt[:, f * FH:(f + 1) * FH], ps[:])
                if b == BS - 1 and m >= NM - 3:
                    # very end of the kernel: quarter the output across four
                    # DMA queues so the drain doesn't trail the compute
                    h = FH // 2
                    for q, eng in enumerate((nc.sync, nc.gpsimd)):
                        cs = f * FH + q * h
                        eng.dma_start(
                            y_d[b, m * P:(m + 1) * P, cs:cs + h],
                            yt[:, cs:cs + h])
                elif b == BS - 1:
                    eng = nc.sync if (2 * m + f) % 2 == 0 else nc.gpsimd
                    eng.dma_start(
                        y_d[b, m * P:(m + 1) * P, f * FH:(f + 1) * FH],
                        yt[:, f * FH:(f + 1) * FH])
                elif f == 1:
                    nc.sync.dma_start(y_d[b, m * P:(m + 1) * P, :], yt[:])
            return emit
        return [mk(m, f) for m in range(NM) for f in range(2)]

    # ---- driver ----
    for t in g_thunks(0):
        t()
    # hold the last two ST(0) chains as extra weave filler for batch 0's
    # pair loop; they land in the earliest pop slots (needed by z pair 0/3)
    # and push full-size G(1) chains into the filler-starved late steps
    held_st = emit_st(0, hold=2)
    held = []
    for b in range(BS):
        filler = (held_st + g_thunks(1)) if b == 0 else held
        emit_pairs(b, filler)
        emit_p(b)
        if b + 2 < BS:
            emit_x_dmas(b + 2)
        if 0 < b < BS - 1:
            for t in g_thunks(b + 1):
                t()
        yth = y_thunks(b)
        if b < BS - 1:
            for t in yth[:-NHELD]:
                t()
            held = yth[-NHELD:]
            emit_st(b + 1)
        else:
            for t in yth:
                t()


_BUILD_CACHE = {}


def build_program():
    if "nc" in _BUILD_CACHE:
        return _BUILD_CACHE["nc"]
    nc = bacc.Bacc("TRN2", target_bir_lowering=False, debug=False,
                   num_devices=NCORES)
    x_d = nc.dram_tensor("x16", [BS, N, C], BF16, kind="ExternalInput").ap()
    xt_d = nc.dram_tensor("xt16", [BS, P, KC * N], BF16,
                          kind="ExternalInput").ap()
    wk_d = nc.dram_tensor("wk", [C, C], BF16, kind="ExternalInput").ap()
    wv_d = nc.dram_tensor("wv", [C, C], BF16, kind="ExternalInput").ap()
    wo_d = nc.dram_tensor("wo", [C, C], BF16, kind="ExternalInput").ap()
    wqt_d = nc.dram_tensor("wqt", [P, KC * C], BF16, kind="ExternalInput").ap()
    y_d = nc.dram_tensor("y", [BS, N, C], F32, kind="ExternalOutput").ap()
    with tile.TileContext(nc) as tc:
        with ExitStack() as ctx:
            _emit(ctx, tc, x_d, xt_d, wk_d, wv_d, wo_d, wqt_d, y_d)
    nc.compile()
    _BUILD_CACHE["nc"] = nc
    return nc


def make_in_maps(x, w_qkv, w_out):
    import ml_dtypes
    bf16 = ml_dtypes.bfloat16
    x = np.asarray(x, dtype=np.float32)
    w_qkv = np.asarray(w_qkv, dtype=np.float32)
    w_out = np.asarray(w_out, dtype=np.float32)

    x16 = np.ascontiguousarray(x.astype(bf16))                    # [B, N, C]
    # xt[b, p, k*N + n] = x[b, n, k*128 + p]
    xt = np.ascontiguousarray(
        x16.transpose(0, 2, 1).reshape(B, KC, P, N)
           .transpose(0, 2, 1, 3).reshape(B, P, KC * N))
    # wqt[p, k*C + c] = w_q[c, k*128 + p]
    wqt = np.ascontiguousarray(
        w_qkv[:, :C].T.reshape(KC, P, C).transpose(1, 0, 2)
                     .reshape(P, KC * C).astype(bf16))
    wk = np.ascontiguousarray(w_qkv[:, C:2 * C].astype(bf16))
    wv = np.ascontiguousarray(w_qkv[:, 2 * C:].astype(bf16))
    wo16 = np.ascontiguousarray(w_out.astype(bf16))
    return [
        {"x16": x16[i * BS:(i + 1) * BS], "xt16": xt[i * BS:(i + 1) * BS],
         "wk": wk, "wv": wv, "wo": wo16, "wqt": wqt}
        for i in range(NCORES)
    ]


def kernel(x, w_qkv, b_qkv=None, w_out=None, b_out=None, **_unused):
    nc = build_program()
    in_maps = make_in_maps(x, w_qkv, w_out)
    res = bass_utils.run_bass_kernel_spmd(nc, in_maps,
                                          core_ids=list(range(NCORES)))
    y = np.concatenate([res.results[i]["y"] for i in range(NCORES)], axis=0)
    return np.asarray(y, dtype=np.float32)


# revision 39
# speedup vs baseline: 1.0264x; 1.0039x over previous
"""ChannelMHSA on Trainium2 (Bass/Tile), data-parallel over batch on 8 cores.

Reference computation (per batch b of x [N, C]):
    qkv  = x @ w_qkv                      # [N, 3C]
    q, k, v per head h: [N, D]
    z_h  = k_h^T @ v_h / sqrt(D)          # [D, D]
    A_h  = softmax(z_h, axis=-1)
    out[n, (h,d)] = sum_e A_h[d, e] q[n, (h,e)]
    y    = out @ w_out                    # [N, C]

Restructured algebra (k, v only ever appear inside z; the output path is
linear in A), which cuts PE work ~25% vs the direct formulation and needs
no on-device transposes:
    G  = x^T @ x                          # [C, C]
    ST = G @ w_k                          # [C, C]   (uses G^T = G)
    z_h  = ST[:, h]^T @ w_v[:, h]         # [D, D] per head, packed in pairs
    A_h  = softmax(z_h / 8)
    R[(h,e), :] = sum_d A_h[d, e] * w_out[(h,d), :]   # block-diag lhsT trick
    P  = w_q @ R                          # [C, C]
    y  = x @ P                            # [N, C]

All matmul operands are bf16 (1 PE cycle/row at any free size, FWL weight
loads, half DMA); PSUM accumulation is fp32; measured rel err ~1e-2 vs the
2e-2 gate.  x^T, w_q^T and all casts are prepared host-side in make_in_maps.

Scheduling: the per-pair softmax (exp on ACT -> reciprocal/block-diag fill
on DVE) has ~1.5-2us of serial latency that would bubble the PE between the
tiny z and R matmuls.  The pair loop therefore takes a "filler" work list:
batch 0 weaves the next batch's G chains in; later batches weave 12
held-back y chains of the previous batch.  A ~3us dummy-matmul prewarm
lifts the PE HAM clock gate to 8/8 before real work arrives.  Inputs and
outputs stream on the Sync hardware-DGE queue (GpSimd's software DGE is
~5x slower and only carries half the batch-0 x load plus part of the
final-tile drain, where queue parallelism still helps).
"""

import sys
from contextlib import ExitStack

import numpy as np

for _p in ("/opt/trn_rl_repo", "/opt/pypackages"):
    if _p not in sys.path:
        sys.path.append(_p)

import concourse.bacc as bacc
import concourse.mybir as mybir
import concourse.tile as tile
from concourse import bass_utils, masks

B, N, C = 32, 1024, 768
H, D = 12, 64
P = 128
NCORES = 8
BS = B // NCORES          # batches per core
KC = C // P               # 6 chunks over C
NM = N // P               # 8 chunks over N
NPAIR = H // 2            # 6 head pairs
F32 = mybir.dt.float32
BF16 = mybir.dt.bfloat16
FH = 384                  # free-dim half of C for matmul tiling
NHELD = 12                # y chains of batch b woven into batch b+1's pairs


def _emit(ctx, tc, x_d, xt_d, wk_d, wv_d, wo_d, wqt_d, y_d):
    nc = tc.nc

    const = ctx.enter_context(tc.tile_pool(name="const", bufs=1))
    xin_pool = ctx.enter_context(tc.tile_pool(name="xin", bufs=2 * NM))
    xt_pool = ctx.enter_context(tc.tile_pool(name="xtp", bufs=3))
    g_pool = ctx.enter_context(tc.tile_pool(name="gp", bufs=KC + 2))
    st_pool = ctx.enter_context(tc.tile_pool(name="stp", bufs=KC + 2))
    r_pool = ctx.enter_context(tc.tile_pool(name="rp", bufs=2))
    p_pool = ctx.enter_context(tc.tile_pool(name="pp", bufs=2))
    y_pool = ctx.enter_context(tc.tile_pool(name="yp", bufs=8))
    sm_pool = ctx.enter_context(tc.tile_pool(name="smp", bufs=8))
    psG = ctx.enter_context(tc.tile_pool(name="psG", bufs=6, space="PSUM"))
    psZ = ctx.enter_context(tc.tile_pool(name="psZ", bufs=2, space="PSUM"))

    ci = [0]

    def copy_out(dst, src):
        # alternate PSUM->SBUF copies between DVE and ACT to balance load
        if ci[0] % 2 == 0:
            nc.vector.tensor_copy(dst, src)
        else:
            nc.scalar.copy(dst, src)
        ci[0] += 1

    # ---- PE pre-warm: ~3us of dummy matmuls on a memset tile so the HAM
    # clock gate reaches 8/8 and the array is hot right as batch-0 x lands;
    # they depend on nothing but the memset so they start with the kernel ----
    warmz = const.tile([P, P], BF16, tag="warmz", name="warmz")
    nc.vector.memset(warmz[:], 0.0)
    wps = psZ.tile([P, P], F32, tag="z", name="warmps", space="PSUM")
    for i in range(30):
        nc.tensor.matmul(wps[:], warmz[:], warmz[:], start=(i == 0),
                         stop=(i == 29))

    # ---- input DMAs; batch 0's x is split finely across both queues to
    # minimize the PE's cold-start wait ----
    xin_b, xt_b = {}, {}

    def emit_x_dmas(b, split=False):
        xin = [xin_pool.tile([P, C], BF16, tag="xin", name=f"xin{b}_{m}")
               for m in range(NM)]
        for m in range(NM):
            if split:
                # the HW-DGE sync queue is faster than GpSimd's SW ring:
                # give it 2/3 of each chunk so both halves finish together
                nc.sync.dma_start(xin[m][:, :512],
                                  x_d[b, m * P:(m + 1) * P, :512])
                nc.gpsimd.dma_start(xin[m][:, 512:],
                                    x_d[b, m * P:(m + 1) * P, 512:])
            else:
                nc.sync.dma_start(xin[m][:], x_d[b, m * P:(m + 1) * P, :])
        xt = xt_pool.tile([P, KC * N], BF16, tag="xt", name=f"xt{b}")
        nc.sync.dma_start(xt[:], xt_d[b])
        xin_b[b], xt_b[b] = xin, xt

    emit_x_dmas(0, split=True)

    # weights ordered by first use: wk (ST), wv (z), wo (R), wqt (P)
    wk, wv, wo = [], [], []
    for k in range(KC):
        t = const.tile([P, C], BF16, tag=f"wk{k}", name=f"wk{k}")
        nc.sync.dma_start(t[:], wk_d[k * P:(k + 1) * P, :])
        wk.append(t)
    for k in range(KC):
        t = const.tile([P, C], BF16, tag=f"wv{k}", name=f"wv{k}")
        nc.sync.dma_start(t[:], wv_d[k * P:(k + 1) * P, :])
        wv.append(t)
    for k in range(KC):
        t = const.tile([P, C], BF16, tag=f"wo{k}", name=f"wo{k}")
        nc.sync.dma_start(t[:], wo_d[k * P:(k + 1) * P, :])
        wo.append(t)
    wqt = const.tile([P, KC * C], BF16, tag="wqt", name="wqt")
    nc.sync.dma_start(wqt[:], wqt_d[:])

    emit_x_dmas(1)

    # Two persistent block-diag lhsT tiles for the R matmul; only the two
    # diagonal [64,64] blocks are rewritten per pair, off-diag zeros persist.
    zeros = const.tile([P, P], F32, tag="zeros", name="zeros")
    nc.vector.memset(zeros[:], 0.0)
    bd_tiles = []
    for i in range(3):
        t = const.tile([P, P], BF16, tag=f"bd{i}", name=f"bd{i}")
        nc.vector.tensor_copy(t[:], zeros[:])
        bd_tiles.append(t)
    ident = const.tile([P, P], BF16, tag="ident", name="ident")
    masks.make_identity(nc, ident[:])

    G_b, ST_b, r_b, p_b = {}, {}, {}, {}

    # ---- phase emitters, shaped as thunk lists so they can be woven ----
    def g_thunks(b):
        """G = x^T x.  Only the upper-triangle blocks (col chunk >= row
        chunk) are computed by matmul; the 15 lower blocks are PE-transposed
        copies of their mirror.  ~40% fewer G cycles."""
        xin = xin_b[b]
        G = [g_pool.tile([P, C], BF16, tag="G", name=f"G{b}_{k}")
             for k in range(KC)]
        G_b[b] = G

        def mk_chain(po, cs, w):
            def emit():
                ps = psG.tile([P, FH], F32, tag="ps",
                              name=f"psg{b}_{po}_{cs}", space="PSUM")
                for m in range(NM):
                    nc.tensor.matmul(
                        ps[:, :w],
                        xin[m][:, po * P:(po + 1) * P],
                        xin[m][:, cs:cs + w],
                        start=(m == 0), stop=(m == NM - 1))
                copy_out(G[po][:, cs:cs + w], ps[:, :w])
            return emit

        def mk_mirror(blocks):
            def emit():
                for po, kk in blocks:
                    tp = psG.tile([P, P], BF16, tag="ps",
                                  name=f"tpg{b}_{po}_{kk}", space="PSUM")
                    nc.tensor.transpose(tp[:], G[kk][:, po * P:(po + 1) * P],
                                        ident[:])
                    copy_out(G[po][:, kk * P:(kk + 1) * P], tp[:])
            return emit

        thunks = []
        for po in range(KC):
            cs = po * P
            while cs < C:
                w = min(FH, C - cs)
                thunks.append(mk_chain(po, cs, w))
                cs += w
        lower = [(po, kk) for po in range(KC) for kk in range(po)]
        for i in range(0, len(lower), 5):
            thunks.append(mk_mirror(lower[i:i + 5]))
        return thunks

    def emit_st(b, hold=0):
        G = G_b.pop(b)
        ST = [st_pool.tile([P, C], BF16, tag="ST", name=f"ST{b}_{k}")
              for k in range(KC)]
        ST_b[b] = ST

        def mk(po, f):
            def emit():
                ps = psG.tile([P, FH], F32, tag="ps", name=f"pss{b}_{po}_{f}",
                              space="PSUM")
                for k in range(KC):
                    nc.tensor.matmul(
                        ps[:],
                        G[k][:, po * P:(po + 1) * P],
                        wk[k][:, f * FH:(f + 1) * FH],
                        start=(k == 0), stop=(k == KC - 1))
                copy_out(ST[po][:, f * FH:(f + 1) * FH], ps[:])
            return emit

        thunks = [mk(po, f) for po in range(KC) for f in range(2)]
        for t in thunks[:len(thunks) - hold]:
            t()
        return thunks[len(thunks) - hold:]

    def emit_pairs(b, filler):
        """z -> softmax -> R per head pair, pulling filler thunks in to keep
        the PE fed while exp/reciprocal run on ACT/DVE."""
        ST = ST_b.pop(b)
        r16 = r_pool.tile([P, KC * C], BF16, tag="r16", name=f"r16{b}")
        r_b[b] = r16
        zps_pair = {}
        for step in range(NPAIR + 2):
            if step == 0 and filler:
                # cover the ST-copy latency at the phase boundary
                filler.pop(0)()
            if step < NPAIR:
                pr = step
                zps = psZ.tile([P, P], F32, tag="z", name=f"z{b}_{pr}",
                               space="PSUM")
                zps_pair[pr] = zps
                for k in range(KC):
                    nc.tensor.matmul(
                        zps[:],
                        ST[k][:, pr * P:(pr + 1) * P],
                        wv[k][:, pr * P:(pr + 1) * P],
                        start=(k == 0), stop=(k == KC - 1))
            # the late steps have little-to-no z work left to hide the
            # softmax serial latency, so give them double filler
            npop = 2 if (step == 0 or step >= NPAIR - 1) else 1
            for _ in range(npop):
                if filler:
                    filler.pop(0)()
            if 1 <= step <= NPAIR:
                # softmax for pair (step-1); its R matmul is deferred one
                # more step so the block-diag LDWEIGHTS never waits on DVE
                pr = step - 1
                zps = zps_pair.pop(pr)
                bdt = bd_tiles[pr % 3]
                ssum = sm_pool.tile([P, 1], F32, tag="ssum", name=f"ss{b}_{pr}")
                aexs = []
                for j in range(2):
                    rb = j * D
                    aex = sm_pool.tile([P, D], F32, tag="aex",
                                       name=f"ae{b}_{pr}_{j}")
                    aexs.append(aex)
                    # softmax needs no max-shift: |z/8| <= ~25, fp32-exp safe
                    nc.scalar.activation(aex[rb:rb + D, :],
                                         zps[rb:rb + D, rb:rb + D],
                                         mybir.ActivationFunctionType.Exp,
                                         bias=0.0, scale=0.125,
                                         accum_out=ssum[rb:rb + D, :])
                rinv = sm_pool.tile([P, 1], F32, tag="rinv",
                                    name=f"ri{b}_{pr}")
                nc.vector.reciprocal(rinv[:], ssum[:])
                for j in range(2):
                    rb = j * D
                    nc.vector.tensor_scalar_mul(bdt[rb:rb + D, rb:rb + D],
                                                aexs[j][rb:rb + D, :],
                                                rinv[rb:rb + D, :])
            if step >= 2:
                pr2 = step - 2
                bdt2 = bd_tiles[pr2 % 3]
                for f in range(2):
                    ps = psG.tile([P, FH], F32, tag="ps",
                                  name=f"psr{b}_{pr2}_{f}", space="PSUM")
                    nc.tensor.matmul(ps[:], bdt2[:],
                                     wo[pr2][:, f * FH:(f + 1) * FH],
                                     start=True, stop=True)
                    # keep the pair phase's copies off ACT (busy with exp)
                    nc.vector.tensor_copy(
                        r16[:, pr2 * C + f * FH:pr2 * C + (f + 1) * FH],
                        ps[:])
        while filler:
            filler.pop(0)()

    def emit_p(b):
        r16 = r_b.pop(b)
        p16 = p_pool.tile([P, KC * C], BF16, tag="p16", name=f"p16{b}")
        p_b[b] = p16
        for po in range(KC):
            for f in range(2):
                ps = psG.tile([P, FH], F32, tag="ps", name=f"psp{b}_{po}_{f}",
                              space="PSUM")
                for k in range(KC):
                    nc.tensor.matmul(
                        ps[:],
                        wqt[:, k * C + po * P:k * C + (po + 1) * P],
                        r16[:, k * C + f * FH:k * C + (f + 1) * FH],
                        start=(k == 0), stop=(k == KC - 1))
                copy_out(p16[:, po * C + f * FH:po * C + (f + 1) * FH], ps[:])

    def y_thunks(b):
        xt, p16 = xt_b[b], p_b[b]
        yts = {}

        def mk(m, f):
            def emit():
                if m not in yts:
                    yts[m] = y_pool.tile([P, C], F32, tag="y", name=f"y{b}_{m}")
                yt = yts[m]
                ps = psG.tile([P, FH], F32, tag="ps", name=f"psy{b}_{m}_{f}",
                              space="PSUM")
                for k in range(KC):
                    nc.tensor.matmul(
                        ps[:],
                        xt[:, k * N + m * P:k * N + (m + 1) * P],
                        p16[:, k * C + f * FH:k * C + (f + 1) * FH],
                        start=(k == 0), stop=(k == KC - 1))
                copy_out(yt[:, f * FH:(f + 1) * FH], ps[:])
                if b == BS - 1 and m >= NM - 3:
                    # very end of the kernel: quarter the output across four
                    # DMA queues so the drain doesn't trail the compute
                    h = FH // 2
                    for q, eng in enumerate((nc.sync, nc.gpsimd)):
                        cs = f * FH + q * h
                        eng.dma_start(
                            y_d[b, m * P:(m + 1) * P, cs:cs + h],
                            yt[:, cs:cs + h])
                elif b == BS - 1:
                    eng = nc.sync if (2 * m + f) % 2 == 0 else nc.gpsimd
                    eng.dma_start(
                        y_d[b, m * P:(m + 1) * P, f * FH:(f + 1) * FH],
                        yt[:, f * FH:(f + 1) * FH])
                elif f == 1:
                    nc.sync.dma_start(y_d[b, m * P:(m + 1) * P, :], yt[:])
            return emit
        return [mk(m, f) for m in range(NM) for f in range(2)]

    # ---- driver ----
    for t in g_thunks(0):
        t()
    # hold the last two ST(0) chains as extra weave filler for batch 0's
    # pair loop; they land in the earliest pop slots (needed by z pair 0/3)
    # and push full-size G(1) chains into the filler-starved late steps
    held_st = emit_st(0, hold=2)
    held = []
    for b in range(BS):
        filler = (held_st + g_thunks(1)) if b == 0 else held
        emit_pairs(b, filler)
        emit_p(b)
        if b + 2 < BS:
            emit_x_dmas(b + 2)
        if 0 < b < BS - 1:
            for t in g_thunks(b + 1):
                t()
        yth = y_thunks(b)
        if b < BS - 1:
            for t in yth[:-NHELD]:
                t()
            held = yth[-NHELD:]
            emit_st(b + 1)
        else:
            for t in yth:
                t()


_BUILD_CACHE = {}


def build_program():
    if "nc" in _BUILD_CACHE:
        return _BUILD_CACHE["nc"]
    nc = bacc.Bacc("TRN2", target_bir_lowering=False, debug=False,
                   num_devices=NCORES)
    x_d = nc.dram_tensor("x16", [BS, N, C], BF16, kind="ExternalInput").ap()
    xt_d = nc.dram_tensor("xt16", [BS, P, KC * N], BF16,
                          kind="ExternalInput").ap()
    wk_d = nc.dram_tensor("wk", [C, C], BF16, kind="ExternalInput").ap()
    wv_d = nc.dram_tensor("wv", [C, C], BF16, kind="ExternalInput").ap()
    wo_d = nc.dram_tensor("wo", [C, C], BF16, kind="ExternalInput").ap()
    wqt_d = nc.dram_tensor("wqt", [P, KC * C], BF16, kind="ExternalInput").ap()
    y_d = nc.dram_tensor("y", [BS, N, C], F32, kind="ExternalOutput").ap()
    with tile.TileContext(nc) as tc:
        with ExitStack() as ctx:
            _emit(ctx, tc, x_d, xt_d, wk_d, wv_d, wo_d, wqt_d, y_d)
    nc.compile()
    _BUILD_CACHE["nc"] = nc
    return nc


def make_in_maps(x, w_qkv, w_out):
    import ml_dtypes
    bf16 = ml_dtypes.bfloat16
    x = np.asarray(x, dtype=np.float32)
    w_qkv = np.asarray(w_qkv, dtype=np.float32)
    w_out = np.asarray(w_out, dtype=np.float32)

    x16 = np.ascontiguousarray(x.astype(bf16))                    # [B, N, C]
    # xt[b, p, k*N + n] = x[b, n, k*128 + p]
    xt = np.ascontiguousarray(
        x16.transpose(0, 2, 1).reshape(B, KC, P, N)
           .transpose(0, 2, 1, 3).reshape(B, P, KC * N))
    # wqt[p, k*C + c] = w_q[c, k*128 + p]
    wqt = np.ascontiguousarray(
        w_qkv[:, :C].T.reshape(KC, P, C).transpose(1, 0, 2)
                     .reshape(P, KC * C).astype(bf16))
    wk = np.ascontiguousarray(w_qkv[:, C:2 * C].astype(bf16))
    wv = np.ascontiguousarray(w_qkv[:, 2 * C:].astype(bf16))
    wo16 = np.ascontiguousarray(w_out.astype(bf16))
    return [
        {"x16": x16[i * BS:(i + 1) * BS], "xt16": xt[i * BS:(i + 1) * BS],
         "wk": wk, "wv": wv, "wo": wo16, "wqt": wqt}
        for i in range(NCORES)
    ]


def kernel(x, w_qkv, b_qkv=None, w_out=None, b_out=None, **_unused):
    nc = build_program()
    in_maps = make_in_maps(x, w_qkv, w_out)
    res = bass_utils.run_bass_kernel_spmd(nc, in_maps,
                                          core_ids=list(range(NCORES)))
    y = np.concatenate([res.results[i]["y"] for i in range(NCORES)], axis=0)
    return np.asarray(y, dtype=np.float32)
